# revision 14
# baseline (speedup 1.0000x reference)
"""Single-head causal self-attention on 8 Trainium2 NeuronCores.

Problem: x[B=8, T=2048, D=2048], Wq/Wk/Wv[D, 128], bq/bk/bv[128]
  q,k,v = x @ W* + b*        (per batch)
  att   = softmax(mask(q k^T / sqrt(128)))
  out   = att @ v            -> [B, T, 128]

Sharding: data-parallel over batch; core b processes batch element b.

Layout strategy: the host supplies x already transposed (xt[d, t]) in
fp16 so each k-tile row block is one contiguous 512KB DMA; weights are
pre-shuffled into their SBUF layout (one contiguous DMA per 4-ktile
piece). The kernel returns the UNNORMALIZED O^T [H, T] plus softmax
row-sums; the host does the final divide and transpose in fp32.

Schedule: interleaved. For chunk c (512 t-columns):
  48 projection matmuls (fp16, N=512, fp32 PSUM accumulate over D)
  ACT evacuates Q^T/K^T (+bias) into persistent SBUF, V^T into a
    staging tile; XBAR DMA transposes V^T -> natural V (no PE cost)
  attention block j=c (needs only chunks <= c):
    S^T = matmul(lhsT=K^T tile, rhs=Q^T range)  narrowed on diagonal
    128-wide triangle mask added in PSUM (DVE), dead P cols zeroed by
    gpsimd, P^T = exp(S^T/sqrt(H)) via ACT -> fp16,
    O^T += matmul(lhsT=V tile, rhs=P^T), rowsum += matmul(lhsT=ones)
    O^T evacuated by DVE, rowsum row 0 by DVE, DMA'd out raw.

Engine discipline (both DMA rings execute on real engines, in order):
  sync ring: all bulk input DMAs up-front, then per-segment XBAR
    transposes + output DMAs. scalar ring: tiny constants only, so the
    ACT engine runs nothing but evacuations and exps — in attention
    blocks ACT is co-critical with the PE, any ring work there stalls
    the S pipeline. PSUM: q/k/v accum (3) + S (3) + O (1) + R (1) = 8.
"""
from contextlib import ExitStack

import numpy as np

import concourse.bacc as bacc
import concourse.bass as bass
import concourse.mybir as mybir
import concourse.tile as tile
from concourse.bass_utils import run_bass_kernel_spmd

B, T, D, H = 8, 2048, 2048, 128
KT = D // 128          # 16 contraction k-tiles for the projections
CH = 512               # t-chunk width == q-range width
NCH = T // CH
SCALE = 1.0 / np.sqrt(np.float32(H))
MASK_NEG = -1.0e4

FP32 = mybir.dt.float32
FP16 = mybir.dt.float16
LOWP = FP16
AF = mybir.ActivationFunctionType

_CACHE = {}


def build():
    nc = bacc.Bacc()
    # xt[c, d, tc] = x[c*CH + tc, d]: host-transposed, chunk-major so a
    # [128, CH] (d-tile, chunk) slice is one contiguous 128KB transfer
    # delivered in exact consumption order
    xt = nc.declare_dram_parameter("xt", [NCH, D, CH], FP16, isOutput=False)
    # w[piece, p, i*4+k, h] = W_i[(piece*4+k)*128 + p, h]: SBUF layout,
    # one contiguous 384KB DMA per piece
    w = nc.declare_dram_parameter("w", [4, 128, 12, H], FP16, isOutput=False)
    bqkv = nc.declare_dram_parameter("bqkv", [H, 3], FP32, isOutput=False)
    c_ones = nc.declare_dram_parameter("c_ones", [128, 128], LOWP,
                                       isOutput=False)
    c_mask = nc.declare_dram_parameter("c_mask", [128, 128], FP32,
                                       isOutput=False)
    out_t = nc.declare_dram_parameter("out_t", [H, T], FP32, isOutput=True)
    out_r = nc.declare_dram_parameter("out_r", [NCH, CH], FP32, isOutput=True)

    with tile.TileContext(nc) as tc, ExitStack() as octx:
        persist = octx.enter_context(tc.tile_pool(name="persist", bufs=1))
        vt_pool = octx.enter_context(tc.tile_pool(name="vt", bufs=2))
        pp = octx.enter_context(tc.tile_pool(name="pp", bufs=4))
        os_pool = octx.enter_context(tc.tile_pool(name="os", bufs=2))
        ps_acc = octx.enter_context(
            tc.tile_pool(name="ps_acc", bufs=1, space="PSUM"))
        ps_s = octx.enter_context(
            tc.tile_pool(name="ps_s", bufs=3, space="PSUM"))
        ps_o = octx.enter_context(
            tc.tile_pool(name="ps_o", bufs=1, space="PSUM"))
        ps_r = octx.enter_context(
            tc.tile_pool(name="ps_r", bufs=1, space="PSUM"))

        w_sb = [[None] * KT for _ in range(3)]
        x_tiles = {}

        def load_w_piece(piece):
            wt = persist.tile([128, 12, H], LOWP, tag=f"w_{piece}",
                              name=f"w_{piece}")
            nc.sync.dma_start(wt[:], w[piece])
            for i in range(3):
                for k in range(4):
                    w_sb[i][piece * 4 + k] = wt[:, i * 4 + k, :]

        # x tiles: 128KB contiguous each; even k-tiles on the sync hwdge
        # ring, odd k-tiles on the gpsimd swdge ring (a third DMA ring —
        # the scalar ring must stay free for ACT work)
        def load_x(c, kts):
            for kt in kts:
                t_ = persist.tile([128, CH], LOWP, tag=f"x_{c}_{kt}",
                                  name=f"x_{c}_{kt}")
                eng = nc.sync if kt % 2 == 0 else nc.gpsimd
                eng.dma_start(t_[:], xt[c, kt * 128:(kt + 1) * 128, :])
                x_tiles[(c, kt)] = t_

        load_w_piece(0)
        load_x(0, range(0, 4))
        load_w_piece(1)
        load_x(0, range(4, 8))
        load_w_piece(2)
        load_x(0, range(8, 12))
        load_w_piece(3)
        load_x(0, range(12, 16))
        load_x(1, range(KT))

        # tiny constants on the scalar ring (drains before first evac)
        b_sb = persist.tile([128, 3], FP32, tag="b")
        nc.scalar.dma_start(b_sb[:], bqkv[:])
        ones_sb = persist.tile([128, 128], LOWP, tag="ones")
        nc.scalar.dma_start(ones_sb[:], c_ones[:])
        # triangle mask for the 128-wide diagonal blocks:
        # tri[k, q] = 0 where q >= k else MASK_NEG
        tri = persist.tile([128, 128], FP32, tag="tri")
        nc.scalar.dma_start(tri[:], c_mask[:])

        qt_sb = persist.tile([128, T], LOWP, tag="qt")   # Q^T [h, t]
        kt_sb = persist.tile([128, T], LOWP, tag="kt")   # K^T [h, t]
        v_nat = [persist.tile([128, H], LOWP, tag=f"v{i}", name=f"v_nat{i}")
                 for i in range(KT)]

        LOOK = 3

        for c in range(NCH):
            # ---- projections for chunk c ------------------------------
            q_ps = ps_acc.tile([128, CH], FP32, tag="q_ps", name=f"q_ps{c}")
            k_ps = ps_acc.tile([128, CH], FP32, tag="k_ps", name=f"k_ps{c}")
            v_ps = ps_acc.tile([128, CH], FP32, tag="v_ps", name=f"v_ps{c}")

            c0 = c * CH
            for kt in range(KT):
                st, sp = kt == 0, kt == KT - 1
                rhs = x_tiles[(c, kt)][:]
                for i, acc in ((0, q_ps), (1, k_ps), (2, v_ps)):
                    nc.tensor.matmul(acc[:], w_sb[i][kt], rhs,
                                     start=st, stop=sp)

            nc.scalar.activation(qt_sb[:, c0:c0 + CH], q_ps[:],
                                 AF.Identity, bias=b_sb[:, 0:1])
            nc.scalar.activation(kt_sb[:, c0:c0 + CH], k_ps[:],
                                 AF.Identity, bias=b_sb[:, 1:2])
            vt_sb = vt_pool.tile([128, CH], LOWP, tag="vt_sb",
                                 name=f"vt_sb{c}")
            nc.scalar.activation(vt_sb[:], v_ps[:],
                                 AF.Identity, bias=b_sb[:, 2:3])
            # V^T -> natural V on the DMA XBAR (zero PE cost); issued
            # before the x prefetch so v_nat is ready for this block
            for tb in range(CH // 128):
                nc.sync.dma_start_transpose(
                    v_nat[c * (CH // 128) + tb][:],
                    vt_sb[:, tb * 128:(tb + 1) * 128])
            if c + 2 < NCH:
                load_x(c + 2, range(KT))

            # ---- attention block j = c --------------------------------
            j = c
            kmax = 4 * j + 4
            q0 = j * CH
            o_ps = ps_o.tile([128, CH], FP32, tag="o_ps", name=f"o_ps{j}")
            r_ps = ps_r.tile([128, CH], FP32, tag="r_ps", name=f"r_ps{j}")
            p_sb = [None] * kmax

            def emit_s(kt, j=j, q0=q0, p_sb=p_sb):
                # diagonal block i: columns < i*128 are fully masked;
                # narrow S/exp to [i*128:], memset the dead P cols,
                # and mask only the 128-wide triangle block
                i = kt - 4 * j
                lo = max(i, 0) * 128
                s_ps = ps_s.tile([128, CH], FP32, tag="s_ps",
                                 name=f"s_ps{j}_{kt}")
                nc.tensor.matmul(
                    s_ps[:, lo:], kt_sb[:, kt * 128:(kt + 1) * 128],
                    qt_sb[:, q0 + lo:q0 + CH], start=True, stop=True)
                if i >= 0:
                    nc.vector.tensor_add(s_ps[:, lo:lo + 128],
                                         s_ps[:, lo:lo + 128], tri[:])
                p = pp.tile([128, CH], LOWP, tag="p")
                if lo > 0:
                    nc.vector.memset(p[:, :lo], 0)
                nc.scalar.activation(p[:, lo:], s_ps[:, lo:],
                                     AF.Exp, scale=SCALE)
                p_sb[kt] = p

            for kt in range(min(LOOK, kmax)):
                emit_s(kt)
            for kt in range(kmax):
                if kt + LOOK < kmax:
                    emit_s(kt + LOOK)
                st, sp = kt == 0, kt == kmax - 1
                nc.tensor.matmul(o_ps[:], v_nat[kt][:], p_sb[kt][:],
                                 start=st, stop=sp)
                nc.tensor.matmul(r_ps[:], ones_sb[:], p_sb[kt][:],
                                 start=st, stop=sp)
                p_sb[kt] = None

            osb = os_pool.tile([128, CH], FP32, tag="osb", name=f"osb{j}")
            nc.vector.tensor_copy(osb[:], o_ps[:])
            nc.sync.dma_start(out_t[:, q0:q0 + CH], osb[:])
            rsb = os_pool.tile([1, CH], FP32, tag="rsb", name=f"rsb{j}")
            nc.vector.tensor_copy(rsb[:], r_ps[0:1, :])
            nc.sync.dma_start(out_r[j], rsb[:])

    nc.finalize()
    return nc


def _get_nc():
    if "nc" not in _CACHE:
        _CACHE["nc"] = build()
    return _CACHE["nc"]


def _consts():
    ones = np.ones((128, 128), dtype=np.float16)
    k_idx = np.arange(128).reshape(128, 1)
    q_idx = np.arange(128).reshape(1, 128)
    mask = np.where(q_idx - k_idx >= 0, 0.0, MASK_NEG).astype(np.float32)
    return {"c_ones": ones, "c_mask": mask}


def kernel(x, Wq, bq, Wk, bk, Wv, bv, _trace=False):
    x = np.asarray(x, dtype=np.float32)
    # w[piece, p, i*4+k, h] = W_i[(piece*4+k)*128 + p, h]
    w3 = np.stack([np.asarray(Wq, np.float32), np.asarray(Wk, np.float32),
                   np.asarray(Wv, np.float32)]).astype(np.float16)
    w = np.ascontiguousarray(
        w3.reshape(3, 4, 4, 128, H).transpose(1, 3, 0, 2, 4)
        .reshape(4, 128, 12, H))
    bqkv = np.stack([np.asarray(bq, np.float32), np.asarray(bk, np.float32),
                     np.asarray(bv, np.float32)], axis=1)
    in_common = {
        "w": w,
        "bqkv": np.ascontiguousarray(bqkv),
        **_consts(),
    }
    nc = _get_nc()
    in_maps = []
    for b in range(B):
        # [NCH, D, CH]: chunk-major transposed fp16 copy of x[b]
        xtb = np.ascontiguousarray(
            x[b].T.reshape(D, NCH, CH).transpose(1, 0, 2), dtype=np.float16)
        in_maps.append(dict(in_common, xt=xtb))
    res = run_bass_kernel_spmd(nc, in_maps, core_ids=list(range(B)),
                               trace=_trace)
    outs = []
    for b in range(B):
        ot = res.results[b]["out_t"]            # [H, T] unnormalized
        r = res.results[b]["out_r"].reshape(1, T)
        outs.append((ot / r).T)
    out = np.stack(outs, axis=0).astype(np.float32)
    if _trace:
        _CACHE["last_exec_time_ns"] = res.exec_time_ns
        _CACHE["last_results"] = res
    return out


# revision 17
# speedup vs baseline: 1.1417x; 1.1417x over previous
"""Single-head causal self-attention on 8 Trainium2 NeuronCores.

Problem: x[B=8, T=2048, D=2048], Wq/Wk/Wv[D, 128], bq/bk/bv[128]
  q,k,v = x @ W* + b*        (per batch)
  att   = softmax(mask(q k^T / sqrt(128)))
  out   = att @ v            -> [B, T, 128]

Sharding: data-parallel over batch; core b processes batch element b.

Layout strategy: the host supplies x already transposed (xt[d, t]) in
fp16 so each k-tile row block is one contiguous 512KB DMA; weights are
pre-shuffled into their SBUF layout (one contiguous DMA per 4-ktile
piece). The kernel returns the UNNORMALIZED O^T [H, T] plus softmax
row-sums; the host does the final divide and transpose in fp32.

Schedule: interleaved. For chunk c (512 t-columns):
  48 projection matmuls (fp16, N=512, fp32 PSUM accumulate over D)
  ACT evacuates Q^T/K^T (+bias) into persistent SBUF, V^T into a
    staging tile; XBAR DMA transposes V^T -> natural V (no PE cost)
  attention block j=c (needs only chunks <= c):
    S^T = matmul(lhsT=K^T tile, rhs=Q^T range)  narrowed on diagonal
    128-wide triangle mask added in PSUM (DVE), dead P cols zeroed by
    gpsimd, P^T = exp(S^T/sqrt(H)) via ACT -> fp16,
    O^T += matmul(lhsT=V tile, rhs=P^T), rowsum += matmul(lhsT=ones)
    O^T evacuated by DVE, rowsum row 0 by DVE, DMA'd out raw.

Engine discipline (both DMA rings execute on real engines, in order):
  sync ring: all bulk input DMAs up-front, then per-segment XBAR
    transposes + output DMAs. scalar ring: tiny constants only, so the
    ACT engine runs nothing but evacuations and exps — in attention
    blocks ACT is co-critical with the PE, any ring work there stalls
    the S pipeline. PSUM: q/k/v accum (3) + S (3) + O (1) + R (1) = 8.
"""
from contextlib import ExitStack

import numpy as np

import concourse.bacc as bacc
import concourse.bass as bass
import concourse.mybir as mybir
import concourse.tile as tile
from concourse.bass_utils import run_bass_kernel_spmd

B, T, D, H = 8, 2048, 2048, 128
KT = D // 128          # 16 contraction k-tiles for the projections
CH = 512               # t-chunk width == q-range width
NCH = T // CH
SCALE = 1.0 / np.sqrt(np.float32(H))
MASK_NEG = -1.0e4

FP32 = mybir.dt.float32
FP16 = mybir.dt.float16
LOWP = FP16
AF = mybir.ActivationFunctionType

_CACHE = {}


def build():
    nc = bacc.Bacc()
    # xt[c, d, tc] = x[c*CH + tc, d]: host-transposed, chunk-major so a
    # [128, CH] (d-tile, chunk) slice is one contiguous 128KB transfer
    # delivered in exact consumption order
    xt = nc.declare_dram_parameter("xt", [NCH, D, CH], FP16, isOutput=False)
    # w[piece, p, i*4+k, h] = W_i[(piece*4+k)*128 + p, h]: SBUF layout,
    # one contiguous 384KB DMA per piece
    w = nc.declare_dram_parameter("w", [4, 128, 12, H], FP16, isOutput=False)
    bqkv = nc.declare_dram_parameter("bqkv", [H, 3], FP32, isOutput=False)
    c_ones = nc.declare_dram_parameter("c_ones", [128, 128], LOWP,
                                       isOutput=False)
    c_mask = nc.declare_dram_parameter("c_mask", [128, 128], FP32,
                                       isOutput=False)
    out_t = nc.declare_dram_parameter("out_t", [H, T], FP32, isOutput=True)
    out_r = nc.declare_dram_parameter("out_r", [NCH, CH], FP32, isOutput=True)

    with tile.TileContext(nc) as tc, ExitStack() as octx:
        persist = octx.enter_context(tc.tile_pool(name="persist", bufs=1))
        vt_pool = octx.enter_context(tc.tile_pool(name="vt", bufs=2))
        pp = octx.enter_context(tc.tile_pool(name="pp", bufs=6))
        os_pool = octx.enter_context(tc.tile_pool(name="os", bufs=2))
        ps_acc = octx.enter_context(
            tc.tile_pool(name="ps_acc", bufs=1, space="PSUM"))
        ps_s = octx.enter_context(
            tc.tile_pool(name="ps_s", bufs=3, space="PSUM"))
        ps_o = octx.enter_context(
            tc.tile_pool(name="ps_o", bufs=1, space="PSUM"))
        ps_r = octx.enter_context(
            tc.tile_pool(name="ps_r", bufs=1, space="PSUM"))

        w_sb = [[None] * KT for _ in range(3)]
        x_tiles = {}

        def load_w_piece(piece):
            wt = persist.tile([128, 12, H], LOWP, tag=f"w_{piece}",
                              name=f"w_{piece}")
            nc.sync.dma_start(wt[:], w[piece])
            for i in range(3):
                for k in range(4):
                    w_sb[i][piece * 4 + k] = wt[:, i * 4 + k, :]

        # x tiles: 128KB contiguous each, all on the sync ring in exact
        # consumption order (the scalar ring must stay free for ACT
        # work, and swdge/gpsimd transfers deliver too slowly)
        def load_x(c, kts):
            for kt in kts:
                t_ = persist.tile([128, CH], LOWP, tag=f"x_{c}_{kt}",
                                  name=f"x_{c}_{kt}")
                nc.sync.dma_start(t_[:], xt[c, kt * 128:(kt + 1) * 128, :])
                x_tiles[(c, kt)] = t_

        load_w_piece(0)
        load_x(0, range(0, 4))
        load_w_piece(1)
        load_x(0, range(4, 8))
        load_w_piece(2)
        load_x(0, range(8, 12))
        load_w_piece(3)
        load_x(0, range(12, 16))
        load_x(1, range(KT))

        # tiny constants on the scalar ring (drains before first evac)
        b_sb = persist.tile([128, 3], FP32, tag="b")
        nc.scalar.dma_start(b_sb[:], bqkv[:])
        ones_sb = persist.tile([128, 128], LOWP, tag="ones")
        nc.scalar.dma_start(ones_sb[:], c_ones[:])
        # triangle mask for the 128-wide diagonal blocks:
        # tri[k, q] = 0 where q >= k else MASK_NEG
        tri = persist.tile([128, 128], FP32, tag="tri")
        nc.scalar.dma_start(tri[:], c_mask[:])

        qt_sb = persist.tile([128, T], LOWP, tag="qt")   # Q^T [h, t]
        kt_sb = persist.tile([128, T], LOWP, tag="kt")   # K^T [h, t]
        v_nat = [persist.tile([128, H], LOWP, tag=f"v{i}", name=f"v_nat{i}")
                 for i in range(KT)]

        LOOK = 3

        for c in range(NCH):
            # ---- projections for chunk c ------------------------------
            q_ps = ps_acc.tile([128, CH], FP32, tag="q_ps", name=f"q_ps{c}")
            k_ps = ps_acc.tile([128, CH], FP32, tag="k_ps", name=f"k_ps{c}")
            v_ps = ps_acc.tile([128, CH], FP32, tag="v_ps", name=f"v_ps{c}")

            c0 = c * CH
            for kt in range(KT):
                st, sp = kt == 0, kt == KT - 1
                rhs = x_tiles[(c, kt)][:]
                for i, acc in ((0, q_ps), (1, k_ps), (2, v_ps)):
                    nc.tensor.matmul(acc[:], w_sb[i][kt], rhs,
                                     start=st, stop=sp)

            nc.scalar.activation(qt_sb[:, c0:c0 + CH], q_ps[:],
                                 AF.Identity, bias=b_sb[:, 0:1])
            nc.scalar.activation(kt_sb[:, c0:c0 + CH], k_ps[:],
                                 AF.Identity, bias=b_sb[:, 1:2])
            vt_sb = vt_pool.tile([128, CH], LOWP, tag="vt_sb",
                                 name=f"vt_sb{c}")
            nc.scalar.activation(vt_sb[:], v_ps[:],
                                 AF.Identity, bias=b_sb[:, 2:3])
            # V^T -> natural V on the DMA XBAR (zero PE cost); issued
            # before the x prefetch so v_nat is ready for this block
            for tb in range(CH // 128):
                nc.sync.dma_start_transpose(
                    v_nat[c * (CH // 128) + tb][:],
                    vt_sb[:, tb * 128:(tb + 1) * 128])
            if c + 2 < NCH:
                load_x(c + 2, range(KT))

            # ---- attention block j = c --------------------------------
            j = c
            kmax = 4 * j + 4
            q0 = j * CH
            o_ps = ps_o.tile([128, CH], FP32, tag="o_ps", name=f"o_ps{j}")
            r_ps = ps_r.tile([128, CH], FP32, tag="r_ps", name=f"r_ps{j}")
            p_sb = [None] * kmax

            def emit_s(kt, j=j, q0=q0, p_sb=p_sb):
                # diagonal block i: columns < i*128 are fully masked;
                # narrow S/exp to [i*128:], memset the dead P cols,
                # and mask only the 128-wide triangle block
                i = kt - 4 * j
                lo = max(i, 0) * 128
                s_ps = ps_s.tile([128, CH], FP32, tag="s_ps",
                                 name=f"s_ps{j}_{kt}")
                nc.tensor.matmul(
                    s_ps[:, lo:], kt_sb[:, kt * 128:(kt + 1) * 128],
                    qt_sb[:, q0 + lo:q0 + CH], start=True, stop=True)
                if i >= 0:
                    nc.vector.tensor_add(s_ps[:, lo:lo + 128],
                                         s_ps[:, lo:lo + 128], tri[:])
                p = pp.tile([128, CH], LOWP, tag="p")
                if lo > 0:
                    nc.vector.memset(p[:, :lo], 0)
                nc.scalar.activation(p[:, lo:], s_ps[:, lo:],
                                     AF.Exp, scale=SCALE)
                p_sb[kt] = p

            # kt order: a few non-diagonal warmups, then the 4 diagonal
            # k-tiles (long S->mask->exp chains, and v_nat[j] lands a few
            # us into the block), then the smooth non-diagonal rest — so
            # the block never ENDS on the latency-heavy diagonal chain
            if j == 0:
                order = list(range(kmax))
            else:
                order = [0, 1, 2, 3] + list(range(4 * j, kmax)) \
                    + list(range(4, 4 * j))
            for idx in range(min(LOOK, kmax)):
                emit_s(order[idx])
            for idx, kt in enumerate(order):
                if idx + LOOK < kmax:
                    emit_s(order[idx + LOOK])
                st, sp = idx == 0, idx == kmax - 1
                nc.tensor.matmul(o_ps[:], v_nat[kt][:], p_sb[kt][:],
                                 start=st, stop=sp)
                nc.tensor.matmul(r_ps[:], ones_sb[:], p_sb[kt][:],
                                 start=st, stop=sp)
                p_sb[kt] = None

            osb = os_pool.tile([128, CH], FP32, tag="osb", name=f"osb{j}")
            nc.vector.tensor_copy(osb[:], o_ps[:])
            nc.sync.dma_start(out_t[:, q0:q0 + CH], osb[:])
            rsb = os_pool.tile([1, CH], FP32, tag="rsb", name=f"rsb{j}")
            nc.vector.tensor_copy(rsb[:], r_ps[0:1, :])
            nc.sync.dma_start(out_r[j], rsb[:])

    nc.finalize()
    return nc


def _get_nc():
    if "nc" not in _CACHE:
        _CACHE["nc"] = build()
    return _CACHE["nc"]


def _consts():
    ones = np.ones((128, 128), dtype=np.float16)
    k_idx = np.arange(128).reshape(128, 1)
    q_idx = np.arange(128).reshape(1, 128)
    mask = np.where(q_idx - k_idx >= 0, 0.0, MASK_NEG).astype(np.float32)
    return {"c_ones": ones, "c_mask": mask}


def kernel(x, Wq, bq, Wk, bk, Wv, bv, _trace=False):
    x = np.asarray(x, dtype=np.float32)
    # w[piece, p, i*4+k, h] = W_i[(piece*4+k)*128 + p, h]
    w3 = np.stack([np.asarray(Wq, np.float32), np.asarray(Wk, np.float32),
                   np.asarray(Wv, np.float32)]).astype(np.float16)
    w = np.ascontiguousarray(
        w3.reshape(3, 4, 4, 128, H).transpose(1, 3, 0, 2, 4)
        .reshape(4, 128, 12, H))
    bqkv = np.stack([np.asarray(bq, np.float32), np.asarray(bk, np.float32),
                     np.asarray(bv, np.float32)], axis=1)
    in_common = {
        "w": w,
        "bqkv": np.ascontiguousarray(bqkv),
        **_consts(),
    }
    nc = _get_nc()
    in_maps = []
    for b in range(B):
        # [NCH, D, CH]: chunk-major transposed fp16 copy of x[b]
        xtb = np.ascontiguousarray(
            x[b].T.reshape(D, NCH, CH).transpose(1, 0, 2), dtype=np.float16)
        in_maps.append(dict(in_common, xt=xtb))
    res = run_bass_kernel_spmd(nc, in_maps, core_ids=list(range(B)),
                               trace=_trace)
    outs = []
    for b in range(B):
        ot = res.results[b]["out_t"]            # [H, T] unnormalized
        r = res.results[b]["out_r"].reshape(1, T)
        outs.append((ot / r).T)
    out = np.stack(outs, axis=0).astype(np.float32)
    if _trace:
        _CACHE["last_exec_time_ns"] = res.exec_time_ns
        _CACHE["last_results"] = res
    return out
